# revision 19
# baseline (speedup 1.0000x reference)
"""Trainium2 Bass kernel for nn_LocalEncoder (masked GRU + attention pooling).

Strategy (v2):
- Data-parallel over batch: 8 cores x 512 rows. Rows are length-sorted and
  dealt round-robin so every core gets an identical length profile, then
  split into 4 chunks of 128 (short->long). Chunk c only scans T_c steps
  (T_c = max length in chunk, uniform across cores) - ~25% less work.
- Feature-major [U partitions, batch free]. All matmuls bf16 -> fp32 PSUM.
- Scan: per chunk-step one PSUM bank holds [z|r|xh|rh]. x-side projections
  (with bias + mask rows folded in) are emitted one step ahead so the PE
  always has ready work; recurrent matmuls accumulate into the same bank.
  s = xh + r*rh via PE identity-accumulate. Trailing-padding mask folded
  via -40 * (1-m) row into the z-gate (freezes h exactly).
- All state stays in SBUF (no DRAM spill): stage[c] = [100, T_c, BC] bf16,
  written in place each step, read directly by the attention phase.
- Attention: sigmoid(A1*last + A2*state_t) with the A1 term applied for
  ALL t; host subtracts the closed-form correction for masked steps and
  adds the contribution of steps beyond T_c. 2-step groups; overlapped
  into the scan tail by reusing finished chunks' PSUM pools.
"""
import sys
sys.path.insert(0, "/opt/trn_rl_repo")
from contextlib import ExitStack

import numpy as np
import ml_dtypes

import concourse.bass as bass
import concourse.bacc as bacc
import concourse.tile as tile
from concourse import mybir
from concourse import bass_utils

bf16 = ml_dtypes.bfloat16
AF = mybir.ActivationFunctionType
OP = mybir.AluOpType

B, T, E, U = 4096, 200, 100, 100
NCORES = 8
NCHUNK = 4
BC = 128
PERCORE = NCHUNK * BC

_CACHE = {}


def _ceil8(x):
    return min(((int(x) + 7) // 8) * 8, T)


def _build(Ts):
    """Ts: per-chunk step counts (uniform across cores)."""
    nc = bacc.Bacc()
    dt = mybir.dt

    xcs = [nc.dram_tensor(f"xc{c}", [128, Ts[c], BC], dt.bfloat16,
                          kind="ExternalInput") for c in range(NCHUNK)]
    wKz = nc.dram_tensor("wKz", [128, U], dt.bfloat16, kind="ExternalInput")
    wKr = nc.dram_tensor("wKr", [128, U], dt.bfloat16, kind="ExternalInput")
    wKh = nc.dram_tensor("wKh", [128, U], dt.bfloat16, kind="ExternalInput")
    wRz = nc.dram_tensor("wRz", [U, U], dt.bfloat16, kind="ExternalInput")
    wRr = nc.dram_tensor("wRr", [U, U], dt.bfloat16, kind="ExternalInput")
    wRh = nc.dram_tensor("wRh", [U, U], dt.bfloat16, kind="ExternalInput")
    wb1h = nc.dram_tensor("wb1h", [1, U], dt.bfloat16, kind="ExternalInput")
    wb1hT = nc.dram_tensor("wb1hT", [U, 1], dt.bfloat16, kind="ExternalInput")
    wA1 = nc.dram_tensor("wA1", [U, U], dt.bfloat16, kind="ExternalInput")
    wA2 = nc.dram_tensor("wA2", [U, U], dt.bfloat16, kind="ExternalInput")
    wVr = nc.dram_tensor("wVr", [U, U], dt.bfloat16, kind="ExternalInput")
    wI = nc.dram_tensor("wI", [U, U], dt.bfloat16, kind="ExternalInput")
    lastout = [nc.dram_tensor(f"lastc{c}", [U, BC], dt.float32,
                              kind="ExternalOutput") for c in range(NCHUNK)]
    outp = [nc.dram_tensor(f"outp{c}", [U, BC], dt.float32,
                           kind="ExternalOutput") for c in range(NCHUNK)]

    maxT = max(Ts)

    with tile.TileContext(nc) as tc, ExitStack() as octx:
        singles = octx.enter_context(tc.tile_pool(name="singles", bufs=1))
        xpool = octx.enter_context(tc.tile_pool(name="xpool", bufs=2))
        gp = octx.enter_context(tc.tile_pool(name="gp", bufs=3))
        bankp = [octx.enter_context(
            tc.tile_pool(name=f"bankp{c}", bufs=2, space="PSUM"))
            for c in range(NCHUNK)]

        def load_w(dram_w, p):
            # stationary weights padded to 128 output columns (FWL-eligible)
            t_ = singles.tile([p, 128], dt.bfloat16, tag=dram_w.name, name=dram_w.name)
            nc.vector.memset(t_, 0.0)
            nc.sync.dma_start(out=t_[:, 0:U], in_=dram_w[:, :])
            return t_
        Kz, Kr, Kh = load_w(wKz, 128), load_w(wKr, 128), load_w(wKh, 128)
        Rz, Rr, Rh = load_w(wRz, U), load_w(wRr, U), load_w(wRh, U)
        A1, A2, Vr, I100 = load_w(wA1, U), load_w(wA2, U), load_w(wVr, U), load_w(wI, U)
        b1h = singles.tile([1, 128], dt.bfloat16, tag="b1h")
        nc.vector.memset(b1h, 0.0)
        nc.sync.dma_start(out=b1h[:, 0:U], in_=wb1h[:, :])
        b1hT = singles.tile([U, 1], dt.bfloat16, tag="b1hT")
        nc.sync.dma_start(out=b1hT, in_=wb1hT[:, :])
        ones = singles.tile([1, BC], dt.bfloat16, tag="ones")
        nc.vector.memset(ones, 1.0)

        stages = []
        for c in range(NCHUNK):
            st = singles.tile([100, Ts[c], BC], dt.bfloat16, tag=f"stage{c}",
                              name=f"stage{c}")
            stages.append(st)

        xblks = [dict() for _ in range(NCHUNK)]
        banks = [dict() for _ in range(NCHUNK)]
        zrs_t = [None] * NCHUNK
        t1_t = [None] * NCHUNK
        last2 = [None] * NCHUNK

        def issue_xdma(c, k):
            if k * 8 >= Ts[c]:
                return
            xt = xpool.tile([128, 8, BC], dt.bfloat16, tag=f"x{c}", name=f"xb{c}")
            nc.sync.dma_start(out=xt, in_=xcs[c][:, k * 8:(k + 1) * 8, :])
            xblks[c][k] = xt

        def xgroup(s):
            """x-side matmuls for step s, all chunks active at s."""
            act = [c for c in range(NCHUNK) if s < Ts[c]]
            for c in act:
                banks[c][s] = bankp[c].tile([128, 4, BC], dt.float32,
                                            tag=f"b{c}", name=f"bank{c}")
            # NOTE: start=True clears has_written for the WHOLE bank, so only
            # the first write per bank may use it; later writes to any region
            # use start=False (stores where unwritten, accumulates elsewhere).
            for gi, W in ((0, Kz), (1, Kr), (2, Kh)):
                stop = (s == 0) if gi < 2 else False
                for c in act:
                    xt = xblks[c][s // 8][:, s % 8, :]
                    nc.tensor.matmul(banks[c][s][:, gi, :], lhsT=W, rhs=xt,
                                     start=(gi == 0), stop=stop)
            if s == 0:
                # seed rh slot with b1h (later steps fold it in via the t1 STT)
                for c in act:
                    nc.tensor.matmul(banks[c][s][:, 3, :], lhsT=b1h, rhs=ones,
                                     start=False, stop=True)

        def recgroup(t):
            for c in range(NCHUNK):
                if t < 1 or t >= Ts[c]:
                    continue
                h = stages[c][:, t - 1, :]
                bk = banks[c][t]
                nc.tensor.matmul(bk[:, 0, :], lhsT=Rz, rhs=h, start=False, stop=True)
                nc.tensor.matmul(bk[:, 1, :], lhsT=Rr, rhs=h, start=False, stop=True)
                nc.tensor.matmul(bk[:, 3, :], lhsT=Rh, rhs=h, start=False, stop=True)

        def gates1(c, t):
            zrs = gp.tile([100, 2, BC], dt.bfloat16, tag=f"zrs{c}", name=f"zrs{c}")
            nc.scalar.activation(zrs, banks[c][t][0:100, 0:2, :], AF.Sigmoid)
            t1 = gp.tile([100, BC], dt.bfloat16, tag=f"t1{c}", name=f"t1{c}")
            if t == 0:
                nc.vector.tensor_tensor(t1, zrs[:, 1, :], banks[c][t][0:100, 3, :],
                                        OP.mult)
            else:
                # t1 = (rh + b1h) * r  in one DVE op
                nc.vector.scalar_tensor_tensor(
                    t1, banks[c][t][0:100, 3, :], b1hT, zrs[:, 1, :],
                    OP.add, OP.mult)
            zrs_t[c], t1_t[c] = zrs, t1

        def iacc_group(t):
            for c in range(NCHUNK):
                if t >= Ts[c]:
                    continue
                nc.tensor.matmul(banks[c][t][:, 2, :], lhsT=I100, rhs=t1_t[c],
                                 start=False, stop=True)

        def gates2(c, t):
            hh = gp.tile([100, BC], dt.bfloat16, tag=f"hh{c}", name=f"hh{c}")
            nc.scalar.activation(hh, banks[c][t][0:100, 2, :], AF.Tanh)
            stw = stages[c][:, t, :]
            if t == 0:
                nc.vector.tensor_tensor(stw, zrs_t[c][:, 0, :], hh, OP.mult)
            else:
                hprev = stages[c][:, t - 1, :]
                d = gp.tile([100, BC], dt.bfloat16, tag=f"d{c}", name=f"d{c}")
                nc.vector.tensor_tensor(d, hh, hprev, OP.subtract)
                e = gp.tile([100, BC], dt.bfloat16, tag=f"e{c}", name=f"e{c}")
                nc.vector.tensor_tensor(e, zrs_t[c][:, 0, :], d, OP.mult)
                nc.gpsimd.tensor_tensor(stw, hprev, e, OP.add)
            del banks[c][t]

        def finish_scan(c):
            tlast = Ts[c] - 1
            lo = gp.tile([100, BC], dt.float32, tag=f"lo{c}", name=f"lo{c}")
            nc.vector.tensor_copy(lo, stages[c][:, tlast, :])
            nc.sync.dma_start(out=lastout[c][:, :], in_=lo)

        # --- attention: chunk c processed in 2-step groups, reusing its own
        #     finished scan pool for PSUM; accumulator in SBUF fp32 on GpSimd.
        att = {}

        def att_start(c):
            acc = singles.tile([100, 2, BC], mybir.dt.float32, tag=f"accs{c}",
                               name=f"accs{c}")
            nc.vector.memset(acc, 0.0)
            l2 = singles.tile([100, 2, BC], mybir.dt.bfloat16, tag=f"l2{c}",
                              name=f"l2{c}")
            nc.vector.tensor_copy(l2[:, 0, :], stages[c][:, Ts[c] - 1, :])
            nc.vector.tensor_copy(l2[:, 1, :], stages[c][:, Ts[c] - 1, :])
            att[c] = {"g": 0, "n": Ts[c] // 2, "pool": bankp[c],
                      "tag": f"b{c}", "acc": acc, "l2": l2}

        def att_done(c):
            return c in att and att[c]["g"] >= att[c]["n"]

        def att_group(c):
            stt = att[c]
            g = stt["g"]
            if g >= stt["n"]:
                return False
            st2 = stages[c][:, 2 * g:2 * g + 2, :]
            sbal = stt["pool"].tile([128, 4, BC], mybir.dt.float32,
                                    tag=stt["tag"], name=f"sbal{c}")
            nc.tensor.matmul(sbal[:, 0:2, :], lhsT=A2, rhs=st2, start=True, stop=False)
            nc.tensor.matmul(sbal[:, 0:2, :], lhsT=A1, rhs=stt["l2"], start=False,
                             stop=True)
            g2 = gp.tile([100, 2, BC], mybir.dt.bfloat16, tag=f"g{c}", name=f"g{c}")
            nc.scalar.activation(g2, sbal[0:100, 0:2, :], AF.Sigmoid)
            nc.tensor.matmul(sbal[:, 2:4, :], lhsT=Vr, rhs=g2, start=False, stop=True)
            tmp = gp.tile([100, 2, BC], mybir.dt.bfloat16, tag=f"tmp{c}", name=f"tmp{c}")
            nc.vector.tensor_tensor(tmp, sbal[0:100, 2:4, :], st2, OP.mult)
            nc.gpsimd.tensor_tensor(stt["acc"], stt["acc"], tmp, OP.add)
            stt["g"] = g + 1
            if stt["g"] == stt["n"]:
                osum = gp.tile([100, BC], mybir.dt.float32, tag=f"os{c}", name=f"os{c}")
                nc.vector.tensor_tensor(osum, stt["acc"][:, 0, :], stt["acc"][:, 1, :],
                                        OP.add)
                nc.sync.dma_start(out=outp[c][:, :], in_=osum)
            return True

        def att_try_starts(t):
            for c in range(NCHUNK):
                if c not in att and t >= Ts[c]:
                    att_start(c)

        def att_pump(budget):
            for c in range(NCHUNK):
                if budget <= 0:
                    break
                if c in att and not att_done(c):
                    if att_group(c):
                        budget -= 1

        # ---------------- emission ----------------
        for c in range(NCHUNK):
            issue_xdma(c, 0)
            issue_xdma(c, 1)
        xgroup(0)

        for t in range(maxT):
            for c in range(NCHUNK):
                if t % 8 == 0 and t >= 8:
                    issue_xdma(c, t // 8 + 1)
            if t + 1 < maxT:
                xgroup(t + 1)
            recgroup(t)
            for c in range(NCHUNK):
                if t < Ts[c]:
                    gates1(c, t)
            iacc_group(t)
            for c in range(NCHUNK):
                if t < Ts[c]:
                    gates2(c, t)
                    if t == Ts[c] - 1:
                        finish_scan(c)
            att_try_starts(t)
            att_pump(3)

        while not all(att_done(c) for c in range(NCHUNK)):
            att_try_starts(10 ** 9)
            att_pump(4)

    nc.compile()
    return nc


def _prep_weights(kernel_w, rec_kernel, bias_, A1_w, A2_w, v):
    b0, b1 = bias_[0], bias_[1]
    w = {}
    Kz = np.zeros((128, U), np.float32)
    Kz[:E] = -kernel_w[:, :U]
    Kz[100, :] = -40.0
    Kz[101, :] = -(b0[:U] + b1[:U])
    Kr = np.zeros((128, U), np.float32)
    Kr[:E] = kernel_w[:, U:2 * U]
    Kr[101, :] = b0[U:2 * U] + b1[U:2 * U]
    Kh = np.zeros((128, U), np.float32)
    Kh[:E] = kernel_w[:, 2 * U:]
    Kh[101, :] = b0[2 * U:]
    w["wKz"], w["wKr"], w["wKh"] = Kz, Kr, Kh
    w["wRz"] = -rec_kernel[:, :U]
    w["wRr"] = rec_kernel[:, U:2 * U]
    w["wRh"] = rec_kernel[:, 2 * U:]
    w["wb1h"] = b1[2 * U:][None, :]
    w["wb1hT"] = b1[2 * U:][:, None]
    w["wA1"] = A1_w
    w["wA2"] = A2_w
    w["wVr"] = np.broadcast_to(v[0][:, None], (U, U)).copy()
    w["wI"] = np.eye(U, dtype=np.float32)
    return {k: vv.astype(bf16) for k, vv in w.items()}


def kernel(session_hidden, mask, kernel, rec_kernel, bias, A1_w, A2_w, v):
    session_hidden = np.asarray(session_hidden, np.float32)
    mask = np.asarray(mask, np.float32)
    kernel_w = np.asarray(kernel, np.float32)
    rec_kernel = np.asarray(rec_kernel, np.float32)
    bias_ = np.asarray(bias, np.float32)
    A1_w = np.asarray(A1_w, np.float32)
    A2_w = np.asarray(A2_w, np.float32)
    v = np.asarray(v, np.float32)

    lengths = mask.sum(1).astype(np.int64)  # in [1, T]
    order = np.argsort(lengths, kind="stable")
    # deal round-robin: sorted rank i -> core i%8, slot i//8
    slot = np.arange(B) // NCORES
    core = np.arange(B) % NCORES
    perm = np.empty(B, np.int64)
    perm[core * PERCORE + slot] = order  # arranged[core*512+slot] = orig row
    lens_a = lengths[perm]
    lens_sorted = lengths[order]
    Ts = tuple(_ceil8(lens_sorted[NCORES * BC * (c + 1) - 1])
               for c in range(NCHUNK))

    key = Ts
    if key not in _CACHE:
        _CACHE[key] = _build(Ts)
    nc = _CACHE[key]
    _CACHE["nc"] = nc

    w = _prep_weights(kernel_w, rec_kernel, bias_, A1_w, A2_w, v)

    x_a = session_hidden[perm].reshape(NCORES, NCHUNK, BC, T, E)
    m_a = mask[perm].reshape(NCORES, NCHUNK, BC, T)
    in_maps = []
    for k in range(NCORES):
        im = dict(w)
        for c in range(NCHUNK):
            Tc = Ts[c]
            xc = np.zeros((128, Tc, BC), np.float32)
            xc[:E] = x_a[k, c, :, :Tc, :].transpose(2, 1, 0)
            xc[100] = 1.0 - m_a[k, c, :, :Tc].transpose(1, 0)
            xc[101] = 1.0
            im[f"xc{c}"] = xc.astype(bf16)
        in_maps.append(im)

    _CACHE["in_maps"] = in_maps
    res = bass_utils.run_bass_kernel_spmd(nc, in_maps, core_ids=list(range(NCORES)))

    out_dev = np.zeros((B, U), np.float32)
    last = np.zeros((B, U), np.float32)
    for k in range(NCORES):
        r = res.results[k]
        for c in range(NCHUNK):
            sl_ = slice(k * PERCORE + c * BC, k * PERCORE + (c + 1) * BC)
            out_dev[sl_] = np.asarray(r[f"outp{c}"]).T.astype(np.float32)
            last[sl_] = np.asarray(r[f"lastc{c}"]).T.astype(np.float32)

    # host correction: device ran steps [0, T_c) with the A1*last term for all t.
    # truth: masked t in [len, T) contribute sigmoid(A2^T last)@v * last.
    Tc_a = np.tile(np.repeat(np.asarray(Ts, np.float32), BC), NCORES)
    sl_ = last @ A2_w
    c_ = last @ A1_w
    sig = lambda a: 1.0 / (1.0 + np.exp(-a))
    a1 = sig(sl_ + c_) @ v[0]
    a0 = sig(sl_) @ v[0]
    lf = lens_a.astype(np.float32)
    out_a = out_dev - (Tc_a - lf)[:, None] * a1[:, None] * last \
        + (T - lf)[:, None] * a0[:, None] * last

    out = np.empty((B, U), np.float32)
    out[perm] = out_a
    _CACHE["debug"] = dict(out_dev=out_dev, last=last, perm=perm, Ts=Ts,
                           lens_a=lens_a, out_a=out_a)
    return out.astype(np.float32)


# revision 20
# speedup vs baseline: 1.1280x; 1.1280x over previous
"""Trainium2 Bass kernel for nn_LocalEncoder (masked GRU + attention pooling).

Strategy (v2):
- Data-parallel over batch: 8 cores x 512 rows. Rows are length-sorted and
  dealt round-robin so every core gets an identical length profile, then
  split into 4 chunks of 128 (short->long). Chunk c only scans T_c steps
  (T_c = max length in chunk, uniform across cores) - ~25% less work.
- Feature-major [U partitions, batch free]. All matmuls bf16 -> fp32 PSUM.
- Scan: per chunk-step one PSUM bank holds [z|r|xh|rh]. x-side projections
  (with bias + mask rows folded in) are emitted one step ahead so the PE
  always has ready work; recurrent matmuls accumulate into the same bank.
  s = xh + r*rh via PE identity-accumulate. Trailing-padding mask folded
  via -40 * (1-m) row into the z-gate (freezes h exactly).
- All state stays in SBUF (no DRAM spill): stage[c] = [100, T_c, BC] bf16,
  written in place each step, read directly by the attention phase.
- Attention: sigmoid(A1*last + A2*state_t) with the A1 term applied for
  ALL t; host subtracts the closed-form correction for masked steps and
  adds the contribution of steps beyond T_c. 2-step groups; overlapped
  into the scan tail by reusing finished chunks' PSUM pools.
"""
import sys
sys.path.insert(0, "/opt/trn_rl_repo")
from contextlib import ExitStack

import numpy as np
import ml_dtypes

import concourse.bass as bass
import concourse.bacc as bacc
import concourse.tile as tile
from concourse import mybir
from concourse import bass_utils

bf16 = ml_dtypes.bfloat16
AF = mybir.ActivationFunctionType
OP = mybir.AluOpType

B, T, E, U = 4096, 200, 100, 100
NCORES = 8
NCHUNK = 4
BC = 128
PERCORE = NCHUNK * BC

_CACHE = {}


def _ceil8(x):
    return min(((int(x) + 7) // 8) * 8, T)


def _build(Ts):
    """Ts: per-chunk step counts (uniform across cores)."""
    nc = bacc.Bacc()
    dt = mybir.dt

    xcs = [nc.dram_tensor(f"xc{c}", [128, Ts[c], BC], dt.bfloat16,
                          kind="ExternalInput") for c in range(NCHUNK)]
    wKz = nc.dram_tensor("wKz", [128, U], dt.bfloat16, kind="ExternalInput")
    wKr = nc.dram_tensor("wKr", [128, U], dt.bfloat16, kind="ExternalInput")
    wKh = nc.dram_tensor("wKh", [128, U], dt.bfloat16, kind="ExternalInput")
    wRz = nc.dram_tensor("wRz", [U, U], dt.bfloat16, kind="ExternalInput")
    wRr = nc.dram_tensor("wRr", [U, U], dt.bfloat16, kind="ExternalInput")
    wRh = nc.dram_tensor("wRh", [U, U], dt.bfloat16, kind="ExternalInput")
    wb1h = nc.dram_tensor("wb1h", [1, U], dt.bfloat16, kind="ExternalInput")
    wb1hT = nc.dram_tensor("wb1hT", [U, 1], dt.bfloat16, kind="ExternalInput")
    wA1 = nc.dram_tensor("wA1", [U, U], dt.bfloat16, kind="ExternalInput")
    wA2 = nc.dram_tensor("wA2", [U, U], dt.bfloat16, kind="ExternalInput")
    wVr = nc.dram_tensor("wVr", [U, U], dt.bfloat16, kind="ExternalInput")
    wI = nc.dram_tensor("wI", [U, U], dt.bfloat16, kind="ExternalInput")
    lastout = [nc.dram_tensor(f"lastc{c}", [U, BC], dt.float32,
                              kind="ExternalOutput") for c in range(NCHUNK)]
    outp = [nc.dram_tensor(f"outp{c}", [U, BC], dt.float32,
                           kind="ExternalOutput") for c in range(NCHUNK)]

    maxT = max(Ts)

    with tile.TileContext(nc) as tc, ExitStack() as octx:
        singles = octx.enter_context(tc.tile_pool(name="singles", bufs=1))
        xpool = octx.enter_context(tc.tile_pool(name="xpool", bufs=2))
        gp = octx.enter_context(tc.tile_pool(name="gp", bufs=3))
        bankp = [octx.enter_context(
            tc.tile_pool(name=f"bankp{c}", bufs=2, space="PSUM"))
            for c in range(NCHUNK)]

        def load_w(dram_w, p):
            # stationary weights padded to 128 output columns (FWL-eligible)
            t_ = singles.tile([p, 128], dt.bfloat16, tag=dram_w.name, name=dram_w.name)
            nc.vector.memset(t_, 0.0)
            nc.sync.dma_start(out=t_[:, 0:U], in_=dram_w[:, :])
            return t_
        Kz, Kr, Kh = load_w(wKz, 128), load_w(wKr, 128), load_w(wKh, 128)
        Rz, Rr, Rh = load_w(wRz, U), load_w(wRr, U), load_w(wRh, U)
        A1, A2, Vr, I100 = load_w(wA1, U), load_w(wA2, U), load_w(wVr, U), load_w(wI, U)
        b1h = singles.tile([1, 128], dt.bfloat16, tag="b1h")
        nc.vector.memset(b1h, 0.0)
        nc.sync.dma_start(out=b1h[:, 0:U], in_=wb1h[:, :])
        b1hT = singles.tile([U, 1], dt.bfloat16, tag="b1hT")
        nc.sync.dma_start(out=b1hT, in_=wb1hT[:, :])
        ones = singles.tile([1, BC], dt.bfloat16, tag="ones")
        nc.vector.memset(ones, 1.0)

        stages = []
        for c in range(NCHUNK):
            st = singles.tile([100, Ts[c], BC], dt.bfloat16, tag=f"stage{c}",
                              name=f"stage{c}")
            stages.append(st)

        xblks = [dict() for _ in range(NCHUNK)]
        banks = [dict() for _ in range(NCHUNK)]
        zrs_t = [None] * NCHUNK
        t1_t = [None] * NCHUNK
        last2 = [None] * NCHUNK

        def issue_xdma(c, k):
            if k * 8 >= Ts[c]:
                return
            xt = xpool.tile([128, 8, BC], dt.bfloat16, tag=f"x{c}", name=f"xb{c}")
            nc.sync.dma_start(out=xt, in_=xcs[c][:, k * 8:(k + 1) * 8, :])
            xblks[c][k] = xt

        def xgroup(s):
            """x-side matmuls for step s, all chunks active at s."""
            act = [c for c in range(NCHUNK) if s < Ts[c]]
            for c in act:
                banks[c][s] = bankp[c].tile([128, 4, BC], dt.float32,
                                            tag=f"b{c}", name=f"bank{c}")
            # NOTE: start=True clears has_written for the WHOLE bank, so only
            # the first write per bank may use it; later writes to any region
            # use start=False (stores where unwritten, accumulates elsewhere).
            for gi, W in ((0, Kz), (1, Kr), (2, Kh)):
                stop = (s == 0) if gi < 2 else False
                for c in act:
                    xt = xblks[c][s // 8][:, s % 8, :]
                    nc.tensor.matmul(banks[c][s][:, gi, :], lhsT=W, rhs=xt,
                                     start=(gi == 0), stop=stop)
            if s == 0:
                # seed rh slot with b1h (later steps fold it in via the t1 STT)
                for c in act:
                    nc.tensor.matmul(banks[c][s][:, 3, :], lhsT=b1h, rhs=ones,
                                     start=False, stop=True)

        def recgroup(t):
            for c in range(NCHUNK):
                if t < 1 or t >= Ts[c]:
                    continue
                h = stages[c][:, t - 1, :]
                bk = banks[c][t]
                nc.tensor.matmul(bk[:, 0, :], lhsT=Rz, rhs=h, start=False, stop=True)
                nc.tensor.matmul(bk[:, 1, :], lhsT=Rr, rhs=h, start=False, stop=True)
                nc.tensor.matmul(bk[:, 3, :], lhsT=Rh, rhs=h, start=False, stop=True)

        def gates1(c, t):
            zrs = gp.tile([100, 2, BC], dt.bfloat16, tag=f"zrs{c}", name=f"zrs{c}")
            nc.scalar.activation(zrs, banks[c][t][0:100, 0:2, :], AF.Sigmoid)
            t1 = gp.tile([100, BC], dt.bfloat16, tag=f"t1{c}", name=f"t1{c}")
            if t == 0:
                nc.vector.tensor_tensor(t1, zrs[:, 1, :], banks[c][t][0:100, 3, :],
                                        OP.mult)
            else:
                # t1 = (rh + b1h) * r  in one DVE op
                nc.vector.scalar_tensor_tensor(
                    t1, banks[c][t][0:100, 3, :], b1hT, zrs[:, 1, :],
                    OP.add, OP.mult)
            zrs_t[c], t1_t[c] = zrs, t1

        def iacc_group(t):
            for c in range(NCHUNK):
                if t >= Ts[c]:
                    continue
                nc.tensor.matmul(banks[c][t][:, 2, :], lhsT=I100, rhs=t1_t[c],
                                 start=False, stop=True)

        def gates2(c, t):
            hh = gp.tile([100, BC], dt.bfloat16, tag=f"hh{c}", name=f"hh{c}")
            nc.scalar.activation(hh, banks[c][t][0:100, 2, :], AF.Tanh)
            stw = stages[c][:, t, :]
            if t == 0:
                nc.vector.tensor_tensor(stw, zrs_t[c][:, 0, :], hh, OP.mult)
            else:
                hprev = stages[c][:, t - 1, :]
                d = gp.tile([100, BC], dt.bfloat16, tag=f"d{c}", name=f"d{c}")
                nc.vector.tensor_tensor(d, hh, hprev, OP.subtract)
                e = gp.tile([100, BC], dt.bfloat16, tag=f"e{c}", name=f"e{c}")
                nc.vector.tensor_tensor(e, zrs_t[c][:, 0, :], d, OP.mult)
                nc.vector.tensor_tensor(stw, hprev, e, OP.add)
            del banks[c][t]

        def finish_scan(c):
            tlast = Ts[c] - 1
            lo = gp.tile([100, BC], dt.float32, tag=f"lo{c}", name=f"lo{c}")
            nc.vector.tensor_copy(lo, stages[c][:, tlast, :])
            nc.sync.dma_start(out=lastout[c][:, :], in_=lo)

        # --- attention: chunk c processed in 2-step groups, reusing its own
        #     finished scan pool for PSUM; accumulator in SBUF fp32 on GpSimd.
        att = {}

        def att_start(c):
            acc = singles.tile([100, 2, BC], mybir.dt.float32, tag=f"accs{c}",
                               name=f"accs{c}")
            nc.vector.memset(acc, 0.0)
            l2 = singles.tile([100, 2, BC], mybir.dt.bfloat16, tag=f"l2{c}",
                              name=f"l2{c}")
            nc.vector.tensor_copy(l2[:, 0, :], stages[c][:, Ts[c] - 1, :])
            nc.vector.tensor_copy(l2[:, 1, :], stages[c][:, Ts[c] - 1, :])
            att[c] = {"g": 0, "n": Ts[c] // 2, "pool": bankp[c],
                      "tag": f"b{c}", "acc": acc, "l2": l2}

        def att_done(c):
            return c in att and att[c]["g"] >= att[c]["n"]

        def att_group(c):
            stt = att[c]
            g = stt["g"]
            if g >= stt["n"]:
                return False
            st2 = stages[c][:, 2 * g:2 * g + 2, :]
            sbal = stt["pool"].tile([128, 4, BC], mybir.dt.float32,
                                    tag=stt["tag"], name=f"sbal{c}")
            nc.tensor.matmul(sbal[:, 0:2, :], lhsT=A2, rhs=st2, start=True, stop=False)
            nc.tensor.matmul(sbal[:, 0:2, :], lhsT=A1, rhs=stt["l2"], start=False,
                             stop=True)
            g2 = gp.tile([100, 2, BC], mybir.dt.bfloat16, tag=f"g{c}", name=f"g{c}")
            nc.scalar.activation(g2, sbal[0:100, 0:2, :], AF.Sigmoid)
            nc.tensor.matmul(sbal[:, 2:4, :], lhsT=Vr, rhs=g2, start=False, stop=True)
            tmp = gp.tile([100, 2, BC], mybir.dt.bfloat16, tag=f"tmp{c}", name=f"tmp{c}")
            nc.vector.tensor_tensor(tmp, sbal[0:100, 2:4, :], st2, OP.mult)
            nc.gpsimd.tensor_tensor(stt["acc"], stt["acc"], tmp, OP.add)
            stt["g"] = g + 1
            if stt["g"] == stt["n"]:
                osum = gp.tile([100, BC], mybir.dt.float32, tag=f"os{c}", name=f"os{c}")
                nc.vector.tensor_tensor(osum, stt["acc"][:, 0, :], stt["acc"][:, 1, :],
                                        OP.add)
                nc.sync.dma_start(out=outp[c][:, :], in_=osum)
            return True

        def att_try_starts(t):
            for c in range(NCHUNK):
                if c not in att and t >= Ts[c]:
                    att_start(c)

        def att_pump(budget):
            for c in range(NCHUNK):
                if budget <= 0:
                    break
                if c in att and not att_done(c):
                    if att_group(c):
                        budget -= 1

        # ---------------- emission ----------------
        for c in range(NCHUNK):
            issue_xdma(c, 0)
            issue_xdma(c, 1)
        xgroup(0)

        for t in range(maxT):
            for c in range(NCHUNK):
                if t % 8 == 0 and t >= 8:
                    issue_xdma(c, t // 8 + 1)
            if t + 1 < maxT:
                xgroup(t + 1)
            recgroup(t)
            for c in range(NCHUNK):
                if t < Ts[c]:
                    gates1(c, t)
            iacc_group(t)
            for c in range(NCHUNK):
                if t < Ts[c]:
                    gates2(c, t)
                    if t == Ts[c] - 1:
                        finish_scan(c)
            att_try_starts(t)
            att_pump(3)

        while not all(att_done(c) for c in range(NCHUNK)):
            att_try_starts(10 ** 9)
            att_pump(4)

    nc.compile()
    return nc


def _prep_weights(kernel_w, rec_kernel, bias_, A1_w, A2_w, v):
    b0, b1 = bias_[0], bias_[1]
    w = {}
    Kz = np.zeros((128, U), np.float32)
    Kz[:E] = -kernel_w[:, :U]
    Kz[100, :] = -40.0
    Kz[101, :] = -(b0[:U] + b1[:U])
    Kr = np.zeros((128, U), np.float32)
    Kr[:E] = kernel_w[:, U:2 * U]
    Kr[101, :] = b0[U:2 * U] + b1[U:2 * U]
    Kh = np.zeros((128, U), np.float32)
    Kh[:E] = kernel_w[:, 2 * U:]
    Kh[101, :] = b0[2 * U:]
    w["wKz"], w["wKr"], w["wKh"] = Kz, Kr, Kh
    w["wRz"] = -rec_kernel[:, :U]
    w["wRr"] = rec_kernel[:, U:2 * U]
    w["wRh"] = rec_kernel[:, 2 * U:]
    w["wb1h"] = b1[2 * U:][None, :]
    w["wb1hT"] = b1[2 * U:][:, None]
    w["wA1"] = A1_w
    w["wA2"] = A2_w
    w["wVr"] = np.broadcast_to(v[0][:, None], (U, U)).copy()
    w["wI"] = np.eye(U, dtype=np.float32)
    return {k: vv.astype(bf16) for k, vv in w.items()}


def kernel(session_hidden, mask, kernel, rec_kernel, bias, A1_w, A2_w, v):
    session_hidden = np.asarray(session_hidden, np.float32)
    mask = np.asarray(mask, np.float32)
    kernel_w = np.asarray(kernel, np.float32)
    rec_kernel = np.asarray(rec_kernel, np.float32)
    bias_ = np.asarray(bias, np.float32)
    A1_w = np.asarray(A1_w, np.float32)
    A2_w = np.asarray(A2_w, np.float32)
    v = np.asarray(v, np.float32)

    lengths = mask.sum(1).astype(np.int64)  # in [1, T]
    order = np.argsort(lengths, kind="stable")
    # deal round-robin: sorted rank i -> core i%8, slot i//8
    slot = np.arange(B) // NCORES
    core = np.arange(B) % NCORES
    perm = np.empty(B, np.int64)
    perm[core * PERCORE + slot] = order  # arranged[core*512+slot] = orig row
    lens_a = lengths[perm]
    lens_sorted = lengths[order]
    Ts = tuple(_ceil8(lens_sorted[NCORES * BC * (c + 1) - 1])
               for c in range(NCHUNK))

    key = Ts
    if key not in _CACHE:
        _CACHE[key] = _build(Ts)
    nc = _CACHE[key]
    _CACHE["nc"] = nc

    w = _prep_weights(kernel_w, rec_kernel, bias_, A1_w, A2_w, v)

    x_a = session_hidden[perm].reshape(NCORES, NCHUNK, BC, T, E)
    m_a = mask[perm].reshape(NCORES, NCHUNK, BC, T)
    in_maps = []
    for k in range(NCORES):
        im = dict(w)
        for c in range(NCHUNK):
            Tc = Ts[c]
            xc = np.zeros((128, Tc, BC), np.float32)
            xc[:E] = x_a[k, c, :, :Tc, :].transpose(2, 1, 0)
            xc[100] = 1.0 - m_a[k, c, :, :Tc].transpose(1, 0)
            xc[101] = 1.0
            im[f"xc{c}"] = xc.astype(bf16)
        in_maps.append(im)

    _CACHE["in_maps"] = in_maps
    res = bass_utils.run_bass_kernel_spmd(nc, in_maps, core_ids=list(range(NCORES)))

    out_dev = np.zeros((B, U), np.float32)
    last = np.zeros((B, U), np.float32)
    for k in range(NCORES):
        r = res.results[k]
        for c in range(NCHUNK):
            sl_ = slice(k * PERCORE + c * BC, k * PERCORE + (c + 1) * BC)
            out_dev[sl_] = np.asarray(r[f"outp{c}"]).T.astype(np.float32)
            last[sl_] = np.asarray(r[f"lastc{c}"]).T.astype(np.float32)

    # host correction: device ran steps [0, T_c) with the A1*last term for all t.
    # truth: masked t in [len, T) contribute sigmoid(A2^T last)@v * last.
    Tc_a = np.tile(np.repeat(np.asarray(Ts, np.float32), BC), NCORES)
    sl_ = last @ A2_w
    c_ = last @ A1_w
    sig = lambda a: 1.0 / (1.0 + np.exp(-a))
    a1 = sig(sl_ + c_) @ v[0]
    a0 = sig(sl_) @ v[0]
    lf = lens_a.astype(np.float32)
    out_a = out_dev - (Tc_a - lf)[:, None] * a1[:, None] * last \
        + (T - lf)[:, None] * a0[:, None] * last

    out = np.empty((B, U), np.float32)
    out[perm] = out_a
    _CACHE["debug"] = dict(out_dev=out_dev, last=last, perm=perm, Ts=Ts,
                           lens_a=lens_a, out_a=out_a)
    return out.astype(np.float32)


# revision 23
# speedup vs baseline: 1.1945x; 1.0590x over previous
"""Trainium2 Bass kernel for nn_LocalEncoder (masked GRU + attention pooling).

Strategy (v2):
- Data-parallel over batch: 8 cores x 512 rows. Rows are length-sorted and
  dealt round-robin so every core gets an identical length profile, then
  split into 4 chunks of 128 (short->long). Chunk c only scans T_c steps
  (T_c = max length in chunk, uniform across cores) - ~25% less work.
- Feature-major [U partitions, batch free]. All matmuls bf16 -> fp32 PSUM.
- Scan: per chunk-step one PSUM bank holds [z|r|xh|rh]. x-side projections
  (with bias + mask rows folded in) are emitted one step ahead so the PE
  always has ready work; recurrent matmuls accumulate into the same bank.
  s = xh + r*rh via PE identity-accumulate. Trailing-padding mask folded
  via -40 * (1-m) row into the z-gate (freezes h exactly).
- All state stays in SBUF (no DRAM spill): stage[c] = [100, T_c, BC] bf16,
  written in place each step, read directly by the attention phase.
- Attention: sigmoid(A1*last + A2*state_t) with the A1 term applied for
  ALL t; host subtracts the closed-form correction for masked steps and
  adds the contribution of steps beyond T_c. 2-step groups; overlapped
  into the scan tail by reusing finished chunks' PSUM pools.
"""
import sys
sys.path.insert(0, "/opt/trn_rl_repo")
from contextlib import ExitStack

import numpy as np
import ml_dtypes

import concourse.bass as bass
import concourse.bacc as bacc
import concourse.tile as tile
from concourse import mybir
from concourse import bass_utils

bf16 = ml_dtypes.bfloat16
AF = mybir.ActivationFunctionType
OP = mybir.AluOpType

B, T, E, U = 4096, 200, 100, 100
NCORES = 8
NCHUNK = 4
BC = 128
PERCORE = NCHUNK * BC

_CACHE = {}


def _ceil8(x):
    return min(((int(x) + 7) // 8) * 8, T)


def _build(Ts):
    """Ts: per-chunk step counts (uniform across cores)."""
    nc = bacc.Bacc()
    dt = mybir.dt

    xcs = [nc.dram_tensor(f"xc{c}", [128, Ts[c], BC], dt.bfloat16,
                          kind="ExternalInput") for c in range(NCHUNK)]
    wKz = nc.dram_tensor("wKz", [128, U], dt.bfloat16, kind="ExternalInput")
    wKr = nc.dram_tensor("wKr", [128, U], dt.bfloat16, kind="ExternalInput")
    wKh = nc.dram_tensor("wKh", [128, U], dt.bfloat16, kind="ExternalInput")
    wRz = nc.dram_tensor("wRz", [U, U], dt.bfloat16, kind="ExternalInput")
    wRr = nc.dram_tensor("wRr", [U, U], dt.bfloat16, kind="ExternalInput")
    wRh = nc.dram_tensor("wRh", [U, U], dt.bfloat16, kind="ExternalInput")
    wb1h = nc.dram_tensor("wb1h", [1, U], dt.bfloat16, kind="ExternalInput")
    wb1hT = nc.dram_tensor("wb1hT", [U, 1], dt.bfloat16, kind="ExternalInput")
    wA1 = nc.dram_tensor("wA1", [U, U], dt.bfloat16, kind="ExternalInput")
    wA2 = nc.dram_tensor("wA2", [U, U], dt.bfloat16, kind="ExternalInput")
    wVr = nc.dram_tensor("wVr", [U, U], dt.bfloat16, kind="ExternalInput")
    wI = nc.dram_tensor("wI", [U, U], dt.bfloat16, kind="ExternalInput")
    lastout = [nc.dram_tensor(f"lastc{c}", [U, BC], dt.float32,
                              kind="ExternalOutput") for c in range(NCHUNK)]
    outp = [nc.dram_tensor(f"outp{c}", [U, BC], dt.float32,
                           kind="ExternalOutput") for c in range(NCHUNK)]

    maxT = max(Ts)

    with tile.TileContext(nc) as tc, ExitStack() as octx:
        singles = octx.enter_context(tc.tile_pool(name="singles", bufs=1))
        xpool = octx.enter_context(tc.tile_pool(name="xpool", bufs=2))
        gp = octx.enter_context(tc.tile_pool(name="gp", bufs=3))
        bankp = [octx.enter_context(
            tc.tile_pool(name=f"bankp{c}", bufs=2, space="PSUM"))
            for c in range(NCHUNK)]

        def load_w(dram_w, p):
            # stationary weights padded to 128 output columns (FWL-eligible)
            t_ = singles.tile([p, 128], dt.bfloat16, tag=dram_w.name, name=dram_w.name)
            nc.vector.memset(t_, 0.0)
            nc.sync.dma_start(out=t_[:, 0:U], in_=dram_w[:, :])
            return t_
        Kz, Kr, Kh = load_w(wKz, 128), load_w(wKr, 128), load_w(wKh, 128)
        Rz, Rr, Rh = load_w(wRz, U), load_w(wRr, U), load_w(wRh, U)
        A1, A2, Vr, I100 = load_w(wA1, U), load_w(wA2, U), load_w(wVr, U), load_w(wI, U)
        b1h = singles.tile([1, 128], dt.bfloat16, tag="b1h")
        nc.vector.memset(b1h, 0.0)
        nc.sync.dma_start(out=b1h[:, 0:U], in_=wb1h[:, :])
        b1hT = singles.tile([U, 1], dt.bfloat16, tag="b1hT")
        nc.sync.dma_start(out=b1hT, in_=wb1hT[:, :])
        ones = singles.tile([1, BC], dt.bfloat16, tag="ones")
        nc.vector.memset(ones, 1.0)

        stages = []
        for c in range(NCHUNK):
            st = singles.tile([100, Ts[c], BC], dt.bfloat16, tag=f"stage{c}",
                              name=f"stage{c}")
            stages.append(st)

        xblks = [dict() for _ in range(NCHUNK)]
        banks = [dict() for _ in range(NCHUNK)]
        zrs_t = [None] * NCHUNK
        t1_t = [None] * NCHUNK
        last2 = [None] * NCHUNK

        def issue_xdma(c, k):
            if k * 8 >= Ts[c]:
                return
            xt = xpool.tile([128, 8, BC], dt.bfloat16, tag=f"x{c}", name=f"xb{c}")
            nc.sync.dma_start(out=xt, in_=xcs[c][:, k * 8:(k + 1) * 8, :])
            xblks[c][k] = xt

        def xgroup(s):
            """x-side matmuls for step s, all chunks active at s."""
            act = [c for c in range(NCHUNK) if s < Ts[c]]
            for c in act:
                banks[c][s] = bankp[c].tile([128, 4, BC], dt.float32,
                                            tag=f"b{c}", name=f"bank{c}")
            # NOTE: start=True clears has_written for the WHOLE bank, so only
            # the first write per bank may use it; later writes to any region
            # use start=False (stores where unwritten, accumulates elsewhere).
            for gi, W in ((0, Kz), (1, Kr), (2, Kh)):
                stop = (s == 0) if gi < 2 else False
                for c in act:
                    xt = xblks[c][s // 8][:, s % 8, :]
                    nc.tensor.matmul(banks[c][s][:, gi, :], lhsT=W, rhs=xt,
                                     start=(gi == 0), stop=stop)
            if s == 0:
                # seed rh slot with b1h (later steps fold it in via the t1 STT)
                for c in act:
                    nc.tensor.matmul(banks[c][s][:, 3, :], lhsT=b1h, rhs=ones,
                                     start=False, stop=True)

        def recgroup(t):
            for c in range(NCHUNK):
                if t < 1 or t >= Ts[c]:
                    continue
                h = stages[c][:, t - 1, :]
                bk = banks[c][t]
                nc.tensor.matmul(bk[:, 0, :], lhsT=Rz, rhs=h, start=False, stop=True)
                nc.tensor.matmul(bk[:, 1, :], lhsT=Rr, rhs=h, start=False, stop=True)
                nc.tensor.matmul(bk[:, 3, :], lhsT=Rh, rhs=h, start=False, stop=True)

        def gates1(c, t):
            zrs = gp.tile([100, 2, BC], dt.bfloat16, tag=f"zrs{c}", name=f"zrs{c}")
            nc.scalar.activation(zrs, banks[c][t][0:100, 0:2, :], AF.Sigmoid)
            t1 = gp.tile([100, BC], dt.bfloat16, tag=f"t1{c}", name=f"t1{c}")
            if t == 0:
                nc.vector.tensor_tensor(t1, zrs[:, 1, :], banks[c][t][0:100, 3, :],
                                        OP.mult)
            else:
                # t1 = (rh + b1h) * r  in one DVE op
                nc.vector.scalar_tensor_tensor(
                    t1, banks[c][t][0:100, 3, :], b1hT, zrs[:, 1, :],
                    OP.add, OP.mult)
            zrs_t[c], t1_t[c] = zrs, t1

        def iacc_group(t):
            for c in range(NCHUNK):
                if t >= Ts[c]:
                    continue
                nc.tensor.matmul(banks[c][t][:, 2, :], lhsT=I100, rhs=t1_t[c],
                                 start=False, stop=True)

        def gates2(c, t):
            hh = gp.tile([100, BC], dt.bfloat16, tag=f"hh{c}", name=f"hh{c}")
            nc.scalar.activation(hh, banks[c][t][0:100, 2, :], AF.Tanh)
            stw = stages[c][:, t, :]
            if t == 0:
                nc.vector.tensor_tensor(stw, zrs_t[c][:, 0, :], hh, OP.mult)
            else:
                hprev = stages[c][:, t - 1, :]
                d = gp.tile([100, BC], dt.bfloat16, tag=f"d{c}", name=f"d{c}")
                nc.vector.tensor_tensor(d, hh, hprev, OP.subtract)
                e = gp.tile([100, BC], dt.bfloat16, tag=f"e{c}", name=f"e{c}")
                nc.vector.tensor_tensor(e, zrs_t[c][:, 0, :], d, OP.mult)
                nc.vector.tensor_tensor(stw, hprev, e, OP.add)
            del banks[c][t]

        def finish_scan(c):
            tlast = Ts[c] - 1
            lo = gp.tile([100, BC], dt.float32, tag=f"lo{c}", name=f"lo{c}")
            nc.vector.tensor_copy(lo, stages[c][:, tlast, :])
            nc.sync.dma_start(out=lastout[c][:, :], in_=lo)

        # --- attention: chunk c processed in 2-step groups, reusing its own
        #     finished scan pool for PSUM; accumulator in SBUF fp32 on GpSimd.
        att = {}

        def att_start(c):
            acc = singles.tile([100, 2, BC], mybir.dt.float32, tag=f"accs{c}",
                               name=f"accs{c}")
            nc.vector.memset(acc, 0.0)
            # c1 = A1^T last, precomputed once, duplicated for the 2-step slots
            c1p = bankp[c].tile([128, 4, BC], mybir.dt.float32, tag=f"b{c}",
                                name=f"c1p{c}")
            nc.tensor.matmul(c1p[:, 0, :], lhsT=A1, rhs=stages[c][:, Ts[c] - 1, :],
                             start=True, stop=True)
            c1s = singles.tile([100, 2, BC], mybir.dt.bfloat16, tag=f"c1s{c}",
                               name=f"c1s{c}")
            nc.vector.tensor_copy(c1s[:, 0, :], c1p[0:100, 0, :])
            nc.vector.tensor_copy(c1s[:, 1, :], c1p[0:100, 0, :])
            # late chunks alternate between their own pool and an already-idle
            # partner pool so the drain pipeline runs 4 banks deep
            partner = {0: None, 1: 0, 2: 1, 3: 0}[c]
            pools = [(bankp[c], f"b{c}")]
            if partner is not None:
                pools.append((bankp[partner], f"b{partner}"))
            att[c] = {"g": 0, "n": Ts[c] // 2, "pools": pools,
                      "acc": acc, "c1s": c1s}

        def att_done(c):
            return c in att and att[c]["g"] >= att[c]["n"]

        def att_group(c):
            stt = att[c]
            g = stt["g"]
            if g >= stt["n"]:
                return False
            st2 = stages[c][:, 2 * g:2 * g + 2, :]
            pool_, tag_ = stt["pools"][g % len(stt["pools"])]
            sbal = pool_.tile([128, 4, BC], mybir.dt.float32,
                              tag=tag_, name=f"sbal{c}")
            nc.tensor.matmul(sbal[:, 0:2, :], lhsT=A2, rhs=st2, start=True, stop=True)
            g2in = gp.tile([100, 2, BC], mybir.dt.bfloat16, tag=f"gi{c}", name=f"gi{c}")
            nc.vector.tensor_tensor(g2in, sbal[0:100, 0:2, :], stt["c1s"], OP.add)
            g2 = gp.tile([100, 2, BC], mybir.dt.bfloat16, tag=f"g{c}", name=f"g{c}")
            nc.scalar.activation(g2, g2in, AF.Sigmoid)
            nc.tensor.matmul(sbal[:, 2:4, :], lhsT=Vr, rhs=g2, start=False, stop=True)
            tmp = gp.tile([100, 2, BC], mybir.dt.bfloat16, tag=f"tmp{c}", name=f"tmp{c}")
            nc.vector.tensor_tensor(tmp, sbal[0:100, 2:4, :], st2, OP.mult)
            nc.gpsimd.tensor_tensor(stt["acc"], stt["acc"], tmp, OP.add)
            stt["g"] = g + 1
            if stt["g"] == stt["n"]:
                osum = gp.tile([100, BC], mybir.dt.float32, tag=f"os{c}", name=f"os{c}")
                nc.vector.tensor_tensor(osum, stt["acc"][:, 0, :], stt["acc"][:, 1, :],
                                        OP.add)
                nc.sync.dma_start(out=outp[c][:, :], in_=osum)
            return True

        def att_try_starts(t):
            for c in range(NCHUNK):
                if c not in att and t >= Ts[c]:
                    att_start(c)

        def att_pump(budget):
            for c in range(NCHUNK):
                if budget <= 0:
                    break
                if c in att and not att_done(c):
                    if att_group(c):
                        budget -= 1

        # ---------------- emission ----------------
        for c in range(NCHUNK):
            issue_xdma(c, 0)
            issue_xdma(c, 1)
        xgroup(0)

        for t in range(maxT):
            for c in range(NCHUNK):
                if t % 8 == 0 and t >= 8:
                    issue_xdma(c, t // 8 + 1)
            if t + 1 < maxT:
                xgroup(t + 1)
            recgroup(t)
            for c in range(NCHUNK):
                if t < Ts[c]:
                    gates1(c, t)
            iacc_group(t)
            for c in range(NCHUNK):
                if t < Ts[c]:
                    gates2(c, t)
                    if t == Ts[c] - 1:
                        finish_scan(c)
            att_try_starts(t)
            att_pump(3)

        while not all(att_done(c) for c in range(NCHUNK)):
            att_try_starts(10 ** 9)
            att_pump(4)

    nc.compile()
    return nc


def _prep_weights(kernel_w, rec_kernel, bias_, A1_w, A2_w, v):
    b0, b1 = bias_[0], bias_[1]
    w = {}
    Kz = np.zeros((128, U), np.float32)
    Kz[:E] = -kernel_w[:, :U]
    Kz[100, :] = -40.0
    Kz[101, :] = -(b0[:U] + b1[:U])
    Kr = np.zeros((128, U), np.float32)
    Kr[:E] = kernel_w[:, U:2 * U]
    Kr[101, :] = b0[U:2 * U] + b1[U:2 * U]
    Kh = np.zeros((128, U), np.float32)
    Kh[:E] = kernel_w[:, 2 * U:]
    Kh[101, :] = b0[2 * U:]
    w["wKz"], w["wKr"], w["wKh"] = Kz, Kr, Kh
    w["wRz"] = -rec_kernel[:, :U]
    w["wRr"] = rec_kernel[:, U:2 * U]
    w["wRh"] = rec_kernel[:, 2 * U:]
    w["wb1h"] = b1[2 * U:][None, :]
    w["wb1hT"] = b1[2 * U:][:, None]
    w["wA1"] = A1_w
    w["wA2"] = A2_w
    w["wVr"] = np.broadcast_to(v[0][:, None], (U, U)).copy()
    w["wI"] = np.eye(U, dtype=np.float32)
    return {k: vv.astype(bf16) for k, vv in w.items()}


def kernel(session_hidden, mask, kernel, rec_kernel, bias, A1_w, A2_w, v):
    session_hidden = np.asarray(session_hidden, np.float32)
    mask = np.asarray(mask, np.float32)
    kernel_w = np.asarray(kernel, np.float32)
    rec_kernel = np.asarray(rec_kernel, np.float32)
    bias_ = np.asarray(bias, np.float32)
    A1_w = np.asarray(A1_w, np.float32)
    A2_w = np.asarray(A2_w, np.float32)
    v = np.asarray(v, np.float32)

    lengths = mask.sum(1).astype(np.int64)  # in [1, T]
    order = np.argsort(lengths, kind="stable")
    # deal round-robin: sorted rank i -> core i%8, slot i//8
    slot = np.arange(B) // NCORES
    core = np.arange(B) % NCORES
    perm = np.empty(B, np.int64)
    perm[core * PERCORE + slot] = order  # arranged[core*512+slot] = orig row
    lens_a = lengths[perm]
    lens_sorted = lengths[order]
    Ts = tuple(_ceil8(lens_sorted[NCORES * BC * (c + 1) - 1])
               for c in range(NCHUNK))

    key = Ts
    if key not in _CACHE:
        _CACHE[key] = _build(Ts)
    nc = _CACHE[key]
    _CACHE["nc"] = nc

    w = _prep_weights(kernel_w, rec_kernel, bias_, A1_w, A2_w, v)

    x_a = session_hidden[perm].reshape(NCORES, NCHUNK, BC, T, E)
    m_a = mask[perm].reshape(NCORES, NCHUNK, BC, T)
    in_maps = []
    for k in range(NCORES):
        im = dict(w)
        for c in range(NCHUNK):
            Tc = Ts[c]
            xc = np.zeros((128, Tc, BC), np.float32)
            xc[:E] = x_a[k, c, :, :Tc, :].transpose(2, 1, 0)
            xc[100] = 1.0 - m_a[k, c, :, :Tc].transpose(1, 0)
            xc[101] = 1.0
            im[f"xc{c}"] = xc.astype(bf16)
        in_maps.append(im)

    _CACHE["in_maps"] = in_maps
    res = bass_utils.run_bass_kernel_spmd(nc, in_maps, core_ids=list(range(NCORES)))

    out_dev = np.zeros((B, U), np.float32)
    last = np.zeros((B, U), np.float32)
    for k in range(NCORES):
        r = res.results[k]
        for c in range(NCHUNK):
            sl_ = slice(k * PERCORE + c * BC, k * PERCORE + (c + 1) * BC)
            out_dev[sl_] = np.asarray(r[f"outp{c}"]).T.astype(np.float32)
            last[sl_] = np.asarray(r[f"lastc{c}"]).T.astype(np.float32)

    # host correction: device ran steps [0, T_c) with the A1*last term for all t.
    # truth: masked t in [len, T) contribute sigmoid(A2^T last)@v * last.
    Tc_a = np.tile(np.repeat(np.asarray(Ts, np.float32), BC), NCORES)
    sl_ = last @ A2_w
    c_ = last @ A1_w
    sig = lambda a: 1.0 / (1.0 + np.exp(-a))
    a1 = sig(sl_ + c_) @ v[0]
    a0 = sig(sl_) @ v[0]
    lf = lens_a.astype(np.float32)
    out_a = out_dev - (Tc_a - lf)[:, None] * a1[:, None] * last \
        + (T - lf)[:, None] * a0[:, None] * last

    out = np.empty((B, U), np.float32)
    out[perm] = out_a
    _CACHE["debug"] = dict(out_dev=out_dev, last=last, perm=perm, Ts=Ts,
                           lens_a=lens_a, out_a=out_a)
    return out.astype(np.float32)
